# revision 14
# baseline (speedup 1.0000x reference)
"""Trainium2 Bass kernel for nn_CorrelationMapLayer.

reference semantics:
    d1 = bilinear_down28(feature1)            # [B, C, 28, 28]
    d2 = bilinear_down28(feature2)            # [B, C, 28, 28]
    f2_sel[b,c,k] = d2[b, c, y_k, x_k]        # knn gather (y=knn[:,1], x=knn[:,0])
    corr = relu(einsum('bck,bchw->bkhw', f2_sel, d1))
    out  = corr / sum_{h,w} exp(corr) * 10

Kernel structure (v3):
  * inputs are cast to bf16 on the host -> HBM traffic halves (DMA is the
    roofline: ~26 MB/core).
  * f2 branch: full 2D separable premultiply (DVE 2x), h-pair add (packed
    last dim -> DVE 2x), w-pair add (strided -> gpsimd, which is otherwise
    idle), 7 flat PE transposes of the contiguous d2 [c, 784], one-hot
    selection matmul -> d2sel [c, K].
  * f1 branch: NO elementwise work. Raw bf16 tiles feed the correlation
    matmul in the original 56x56 space; the bilinear downsample is applied
    AFTER the matmul on corr56 [K=100, 3136] (K < C so this is ~2.3x
    cheaper): ACT copies psum->bf16, premultiply by the same separable
    weight map (DVE 2x), h-pair add (2x), strided w-pair add -> craw,
    one relu, exp+accumulate, reciprocal, scale.
  * Data parallel over batch: 4 batches per core x 8 cores.
"""

import os
import sys

import numpy as np

for _p in (
    "/root/.axon_site",
    "/root/.axon_site/_ro/trn_rl_repo",
    "/root/.axon_site/_ro/pypackages",
    "/opt/trn_rl_repo",
):
    if os.path.isdir(_p) and _p not in sys.path:
        sys.path.append(_p)

import concourse.bacc as bacc
import concourse.mybir as mybir
import concourse.tile as tile
from concourse import bass_utils

F32 = mybir.dt.float32
BF16 = mybir.dt.bfloat16
AF = mybir.ActivationFunctionType

B, C, H, W, K = 32, 512, 56, 56, 100
NCORES = 8
BL = B // NCORES  # batches per core
S = 28
HW = H * W  # 3136
HW28 = S * S  # 784
NCB = C // 128  # 4 channel blocks
NCHUNK = 7  # corr h-row groups (7 x 8 rows)
NK = 112  # knn count padded to a multiple of 16 for ap_gather
# corr psum tiles cover h-row groups: 3 tiles of 16 rows + 1 tile of 8 rows
CORR_TILES = [(0, 16), (16, 16), (32, 16), (48, 8)]

BF16_NP = mybir.dt.np(BF16)


def _bilinear_matrix(in_size: int, out_size: int) -> np.ndarray:
    scale = np.float32((in_size - 1) / (out_size - 1)) if out_size > 1 else np.float32(0)
    coords = np.arange(out_size, dtype=np.float32) * scale
    lo = np.floor(coords).astype(np.int32)
    hi = np.minimum(lo + 1, in_size - 1)
    frac = coords - lo.astype(np.float32)
    M = np.zeros((out_size, in_size), np.float32)
    np.add.at(M, (np.arange(out_size), lo), np.float32(1.0) - frac)
    np.add.at(M, (np.arange(out_size), hi), frac)
    return M


def _tap_weights() -> np.ndarray:
    """wvec[w]: weight applied to input index w, whose (unique) consumer is
    output index w//2. Verifies the 2-tap stride-2 structure exactly."""
    M = _bilinear_matrix(H, S)  # [28, 56]
    wvec = np.zeros(H, np.float32)
    for w in range(H):
        wvec[w] = M[w // 2, w]
    M2 = np.zeros_like(M)
    for ow in range(S):
        M2[ow, 2 * ow] = wvec[2 * ow]
        M2[ow, 2 * ow + 1] = wvec[2 * ow + 1]
    assert np.abs(M - M2).max() <= 1e-6, "bilinear 2-tap structure violated"
    return wvec


_WVEC = _tap_weights()
# WF[p, h*56+w] = wvec[h]*wvec[w]  (full separable 2D weight map)
_WF_ROW = (np.repeat(_WVEC, W) * np.tile(_WVEC, H)).astype(np.float32)
WF_NP = np.ascontiguousarray(
    np.broadcast_to(_WF_ROW[None, :], (128, HW)), dtype=BF16_NP
)
IDENT_NP = np.ascontiguousarray(np.eye(128, dtype=BF16_NP))


def _gather_tables(knn_inds: np.ndarray):
    """ap_gather indices + per-k tap weights.

    tf2 [c, 3136] viewed as pairs [c, 1568, 2]: knn k at (x, y) needs pairs
    56*y + x (row 2y) and 56*y + 28 + x (row 2y+1); the 4 bilinear tap
    weights are folded into the combine as interleaved weight rows."""
    knn = np.asarray(knn_inds)
    idx0 = np.zeros((128, NK // 16), np.int16)
    idx1 = np.zeros((128, NK // 16), np.int16)
    w0 = np.zeros((128, 2 * NK), np.float32)
    w1 = np.zeros((128, 2 * NK), np.float32)
    for k in range(knn.shape[0]):
        x = int(knn[k, 0])
        y = int(knn[k, 1])
        idx0[k % 16 :: 16, k // 16] = 56 * y + x
        idx1[k % 16 :: 16, k // 16] = 56 * y + 28 + x
        w0[:, 2 * k] = _WVEC[2 * y] * _WVEC[2 * x]
        w0[:, 2 * k + 1] = _WVEC[2 * y] * _WVEC[2 * x + 1]
        w1[:, 2 * k] = _WVEC[2 * y + 1] * _WVEC[2 * x]
        w1[:, 2 * k + 1] = _WVEC[2 * y + 1] * _WVEC[2 * x + 1]
    return (
        np.ascontiguousarray(idx0),
        np.ascontiguousarray(idx1),
        np.ascontiguousarray(w0.astype(BF16_NP)),
        np.ascontiguousarray(w1.astype(BF16_NP)),
    )


def _make_in_maps(f1: np.ndarray, f2: np.ndarray, knn_inds: np.ndarray):
    idx0, idx1, w0, w1 = _gather_tables(knn_inds)
    in_maps = []
    for c in range(NCORES):
        in_maps.append(
            {
                "f1": np.ascontiguousarray(f1[c * BL : (c + 1) * BL]),
                "f2": np.ascontiguousarray(f2[c * BL : (c + 1) * BL]),
                "wf": WF_NP,
                "idx0": idx0,
                "idx1": idx1,
                "w0": w0,
                "w1": w1,
            }
        )
    return in_maps


def _build(tc, out_ap, f1_ap, f2_ap, wf_ap, idx0_ap, idx1_ap, w0_ap, w1_ap):
    nc = tc.nc
    MS = __import__("concourse.bass", fromlist=["MemorySpace"]).MemorySpace

    from contextlib import ExitStack

    with ExitStack() as ctx:
        const = ctx.enter_context(tc.tile_pool(name="const", bufs=1))
        tf2p = ctx.enter_context(tc.tile_pool(name="tf2p", bufs=3))
        gp = ctx.enter_context(tc.tile_pool(name="gp", bufs=4))
        mp = ctx.enter_context(tc.tile_pool(name="mp", bufs=4))
        sp2 = ctx.enter_context(tc.tile_pool(name="sp2", bufs=4))
        d2selp = ctx.enter_context(tc.tile_pool(name="d2selp", bufs=8))
        tf1p = ctx.enter_context(tc.tile_pool(name="tf1p", bufs=8))
        cbp = ctx.enter_context(tc.tile_pool(name="cbp", bufs=2))
        up = ctx.enter_context(tc.tile_pool(name="up", bufs=2))
        vp = ctx.enter_context(tc.tile_pool(name="vp", bufs=2))
        crawp = ctx.enter_context(tc.tile_pool(name="crawp", bufs=2))
        c28p = ctx.enter_context(tc.tile_pool(name="c28p", bufs=2))
        expbp = ctx.enter_context(tc.tile_pool(name="expbp", bufs=2))
        obp = ctx.enter_context(tc.tile_pool(name="obp", bufs=2))
        smallp = ctx.enter_context(tc.tile_pool(name="smallp", bufs=6))
        cpsp = ctx.enter_context(tc.tile_pool(name="cpsp", bufs=6, space=MS.PSUM))

        I16 = mybir.dt.int16
        wf = const.tile([128, HW], BF16, tag="wf")
        idx0 = const.tile([128, NK // 16], I16, tag="idx0")
        idx1 = const.tile([128, NK // 16], I16, tag="idx1")
        w0 = const.tile([128, 2 * NK], BF16, tag="w0")
        w1 = const.tile([128, 2 * NK], BF16, tag="w1")
        nc.sync.dma_start(wf[:], wf_ap)
        nc.sync.dma_start(idx0[:], idx0_ap)
        nc.sync.dma_start(idx1[:], idx1_ap)
        nc.sync.dma_start(w0[:], w0_ap)
        nc.sync.dma_start(w1[:], w1_ap)

        for b in range(BL):
            # ---- f2 branch: gather the 4 bilinear taps per knn point
            # straight from the raw tile, weighted combine -> d2sel ----
            d2sel_tiles = []
            for i in range(NCB):
                tf2 = tf2p.tile([128, HW], BF16, tag="tf2")
                nc.sync.dma_start(
                    tf2[:],
                    f2_ap[b, i * 128 : (i + 1) * 128, :, :].rearrange(
                        "c h w -> c (h w)"
                    ),
                )
                g0 = gp.tile([128, 2 * NK], BF16, tag="g0")
                g1 = gp.tile([128, 2 * NK], BF16, tag="g1")
                nc.gpsimd.ap_gather(
                    g0[:], tf2[:], idx0[:],
                    channels=128, num_elems=HW // 2, d=2, num_idxs=NK,
                )
                nc.gpsimd.ap_gather(
                    g1[:], tf2[:], idx1[:],
                    channels=128, num_elems=HW // 2, d=2, num_idxs=NK,
                )
                m0 = mp.tile([128, 2 * NK], BF16, tag="m0")
                nc.vector.tensor_mul(m0[:], g0[:], w0[:])
                m1 = mp.tile([128, 2 * NK], BF16, tag="m1")
                nc.vector.tensor_mul(m1[:], g1[:], w1[:])
                m0v = m0.rearrange("c (k d) -> c k d", d=2)
                m1v = m1.rearrange("c (k d) -> c k d", d=2)
                s01 = sp2.tile([128, NK], BF16, tag="s01")
                nc.vector.tensor_add(s01[:], m0v[:, :, 0], m0v[:, :, 1])
                s23 = sp2.tile([128, NK], BF16, tag="s23")
                nc.vector.tensor_add(s23[:], m1v[:, :, 0], m1v[:, :, 1])
                dsel = d2selp.tile([128, NK], BF16, tag="d2sel")
                nc.vector.tensor_add(dsel[:], s01[:], s23[:])
                d2sel_tiles.append(dsel)

            # ---- f1 loads (raw bf16, no elementwise work) ----
            tf1_tiles = []
            for i in range(NCB):
                tf1 = tf1p.tile([128, HW], BF16, tag="tf1")
                nc.sync.dma_start(
                    tf1[:],
                    f1_ap[b, i * 128 : (i + 1) * 128, :, :].rearrange(
                        "c h w -> c (h w)"
                    ),
                )
                tf1_tiles.append(tf1.rearrange("c (h w) -> c h w", h=H))

            # ---- correlation in 56x56 space + post-matmul downsample ----
            craw = crawp.tile([K, HW28], F32, tag="craw")
            cr3 = craw.rearrange("k (h w) -> k h w", h=S)
            for g in range(NCHUNK):  # 7 groups of 8 h-rows
                h0 = g * 8
                cps = cpsp.tile([K, 8 * W], F32, tag="cps")
                for i in range(NCB):
                    nc.tensor.matmul(
                        cps[:],
                        d2sel_tiles[i][:, 0:K],
                        tf1_tiles[i][:, h0 : h0 + 8, :],
                        start=(i == 0),
                        stop=(i == NCB - 1),
                    )
                cb = cbp.tile([K, 8 * W], BF16, tag="cb")
                if g % 3 == 2:
                    nc.vector.tensor_copy(cb[:], cps[:])
                else:
                    nc.scalar.copy(cb[:], cps[:])
                u = up.tile([K, 8 * W], BF16, tag="u")
                nc.vector.tensor_mul(
                    u[:], cb[:], wf[0:K, h0 * W : (h0 + 8) * W]
                )
                u3 = u.rearrange("k (h w) -> k h w", h=8)
                v = vp.tile([K, 4 * W], BF16, tag="v")
                v3 = v.rearrange("k (h w) -> k h w", h=4)
                nc.vector.tensor_add(v3, u3[:, 0:8:2, :], u3[:, 1:8:2, :])
                nc.vector.tensor_add(
                    cr3[:, g * 4 : (g + 1) * 4, :],
                    v3[:, :, 0:W:2],
                    v3[:, :, 1:W:2],
                )

            # ---- relu, exp + accumulate, reciprocal, scale by 10/denom ----
            c28 = c28p.tile([K, HW28], F32, tag="c28")
            nc.scalar.activation(c28[:], craw[:], AF.Relu)
            expb = expbp.tile([K, HW28], BF16, tag="expb")
            den = smallp.tile([K, 1], F32, tag="den")
            nc.scalar.activation(expb[:], c28[:], AF.Exp, accum_out=den[:])
            rec = smallp.tile([K, 1], F32, tag="rec")
            nc.vector.reciprocal(rec[:], den[:])
            rec10 = smallp.tile([K, 1], F32, tag="rec10")
            nc.vector.tensor_scalar_mul(rec10[:], rec[:], 10.0)
            ob = obp.tile([K, HW28], F32, tag="ob")
            nc.scalar.mul(ob[:], c28[:], rec10[:])
            # out DMA on the ACT HWDGE queue: keeps the SP queue a pure
            # input stream (no head-of-line blocking on the epilogue)
            nc.scalar.dma_start(out_ap[b], ob[:])


_CACHE: dict = {}


def _get_nc():
    if "nc" in _CACHE:
        return _CACHE["nc"]
    nc = bacc.Bacc(
        "TRN2",
        target_bir_lowering=False,
        debug=False,
        enable_asserts=False,
        num_devices=NCORES,
    )
    f1 = nc.dram_tensor("f1", [BL, C, H, W], BF16, kind="ExternalInput").ap()
    f2 = nc.dram_tensor("f2", [BL, C, H, W], BF16, kind="ExternalInput").ap()
    wf = nc.dram_tensor("wf", [128, HW], BF16, kind="ExternalInput").ap()
    idx0 = nc.dram_tensor("idx0", [128, NK // 16], mybir.dt.int16, kind="ExternalInput").ap()
    idx1 = nc.dram_tensor("idx1", [128, NK // 16], mybir.dt.int16, kind="ExternalInput").ap()
    w0 = nc.dram_tensor("w0", [128, 2 * NK], BF16, kind="ExternalInput").ap()
    w1 = nc.dram_tensor("w1", [128, 2 * NK], BF16, kind="ExternalInput").ap()
    out = nc.dram_tensor("out", [BL, K, HW28], F32, kind="ExternalOutput").ap()
    with tile.TileContext(nc) as tc:
        _build(tc, out, f1, f2, wf, idx0, idx1, w0, w1)
    nc.compile()
    _CACHE["nc"] = nc
    return nc


def kernel(feature1, feature2, knn_inds):
    f1 = np.asarray(feature1, dtype=np.float32).astype(BF16_NP)
    f2 = np.asarray(feature2, dtype=np.float32).astype(BF16_NP)
    nc = _get_nc()
    in_maps = _make_in_maps(f1, f2, knn_inds)
    res = bass_utils.run_bass_kernel_spmd(nc, in_maps, core_ids=list(range(NCORES)))
    _CACHE["last_results"] = res
    out = np.concatenate([r["out"] for r in res.results], axis=0)
    return out.reshape(B, K, S, S)
